# revision 26
# baseline (speedup 1.0000x reference)
"""AttnBlock (GroupNorm -> q/k/v 1x1 conv -> spatial softmax attention -> proj -> residual)
for Trainium2, 8 NeuronCores.

Sharding: core i handles batch i//2, query-position chunk i%2 (2048 of 4096 positions).
Each core receives the full image of its batch (needed for GroupNorm stats and full K/V),
computes K/V for all positions (2x duplicated work, ~10% overhead, no collectives needed),
and attention rows for its own query chunk.

Kernel structure (per core), all matmuls in float32r (1 cyc/row at free>=256):
  pass 1: stream x in 512-wide chunks -> bn_stats; cross-partition group reduction
          and group->channel broadcast via tiny PE matmuls against 0/1 indicator
          matrices (no DRAM round trips on the critical path).
  q conv: stream xq chunks, normalize, q = wqT^T @ h_q (scaled by c^-0.5, biased).
  pass 2 (flash-style, k/v never fully materialized): for each 512-wide m-chunk:
          normalize -> k_mc, vT_mc convs; for each 512-wide n-tile:
          sT[m,n] = k^T q (PSUM), P = exp(sT) (ScalarE, PSUM->SBUF),
          PV partial = vT^T @ P accumulated in PSUM over the chunk then added to
          an SBUF accumulator; softmax denominator = DVE-presum of the 4 exp
          tiles followed by a single ones-matmul on PE.
  final:  proj conv runs on the unscaled accumulator (the per-column 1/den commutes
          through the matmul); epilogue = po * (1/den, partition-broadcast) + (x + bo)
          banked in SBUF during the q-conv pass; DMA out.
"""

import math
import os
import sys

sys.path.insert(0, "/opt/trn_rl_repo")

import numpy as np

import concourse.bacc as bacc
import concourse.bass as bass
import concourse.mybir as mybir
import concourse.tile as tile
from concourse.bass_utils import run_bass_kernel_spmd

F32 = mybir.dt.float32
F32R = mybir.dt.float32r
MULT = mybir.AluOpType.mult
ADD = mybir.AluOpType.add
SUB = mybir.AluOpType.subtract
AX = mybir.AxisListType.X
XY = mybir.AxisListType.XY
EXP = mybir.ActivationFunctionType.Exp
SQRT = mybir.ActivationFunctionType.Sqrt

B, C, H, W = 4, 512, 64, 64
HW = H * W              # 4096
G = 32                  # groups
GS = C // G             # 16 channels per group
NQ = HW // 2            # query positions per core
EPS = 1e-5
N_CORES = 8

LAST_RESULTS = None     # BassKernelResults of the most recent run (for profiling)


def _build(dt_mm=F32R):
    CT = C // 128            # 4 channel partition-tiles
    NT = NQ // 512           # 4 n-tiles per core
    MC = HW // 512           # 8 m-chunks
    inv_sqrt_c = 1.0 / math.sqrt(C)

    nc = bacc.Bacc("TRN2", target_bir_lowering=False, debug=False)

    x_img = nc.dram_tensor("x_img", [C, HW], F32R, kind="ExternalInput").ap()
    xq = nc.dram_tensor("xq", [C, NQ], F32R, kind="ExternalInput").ap()
    wts = {
        n: nc.dram_tensor(n, [C, C], F32R, kind="ExternalInput").ap()
        for n in ("wqT", "wkT", "wvT", "woT")
    }
    mg_d = nc.dram_tensor("Mg", [C, G], F32, kind="ExternalInput").ap()
    m2_d = nc.dram_tensor("M2", [G, C], F32, kind="ExternalInput").ap()
    vecs = {
        n: nc.dram_tensor(n, [C], F32, kind="ExternalInput").ap()
        for n in ("bqs", "bk", "bv", "bo", "gamma", "beta")
    }
    out = nc.dram_tensor("out", [C, NQ], F32, kind="ExternalOutput").ap()

    rx = x_img.rearrange("(t p) m -> p t m", p=128)
    rxq = xq.rearrange("(t p) n -> p t n", p=128)
    rout = out.rearrange("(t p) n -> p t n", p=128)

    with tile.TileContext(nc) as tc:
        with (
            tc.tile_pool(name="singles", bufs=1) as singles,
            tc.tile_pool(name="wpool", bufs=1) as wpool,
            tc.tile_pool(name="xs", bufs=2) as xs,
            tc.tile_pool(name="statp", bufs=2) as statp,
            tc.tile_pool(name="kpool", bufs=2) as kpool,
            tc.tile_pool(name="vpool", bufs=2) as vpool,
            tc.tile_pool(name="ppool", bufs=5) as ppool,
            tc.tile_pool(name="opool", bufs=3) as opool,
            tc.tile_pool(name="xrpool", bufs=2) as xrpool,
            tc.tile_pool(name="ps_main", bufs=3, space="PSUM") as ps_main,
            tc.tile_pool(name="ps_ao", bufs=4, space="PSUM") as ps_ao,
            tc.tile_pool(name="ps_den", bufs=1, space="PSUM") as ps_den,
            tc.tile_pool(name="dram", bufs=1, space="DRAM") as dram,
        ):
            # ---- constants / small loads ----
            bias_cols = {}
            for n in ("bqs", "bk", "bo", "gamma", "beta"):
                t = singles.tile([128, CT], F32, tag=f"col_{n}")
                nc.sync.dma_start(out=t, in_=vecs[n].rearrange("(t p) -> p t", p=128))
                bias_cols[n] = t
            # bv replicated across all 128 partitions (vT has m on partitions)
            bv_b = singles.tile([128, C], F32, tag="bv_b")
            bv_src = vecs["bv"]
            nc.sync.dma_start(
                out=bv_b,
                in_=bass.AP(tensor=bv_src.tensor, offset=bv_src.offset,
                            ap=[[0, 128], bv_src.ap[0]]),
            )
            eps_t = singles.tile([G, 1], F32, tag="eps")
            nc.vector.memset(eps_t, EPS)
            ones_f = singles.tile([128, 1], F32, tag="ones_f")
            nc.vector.memset(ones_f, 1.0)
            dummy = singles.tile([1, 1], F32, tag="dummy")
            nc.vector.memset(dummy, 1.0)
            nc.scalar.activation(out=dummy, in_=dummy, func=SQRT)
            nc.scalar.activation(out=dummy, in_=dummy, func=EXP)
            ones_t = singles.tile([128, 1], F32R, tag="ones")
            nc.vector.tensor_copy(out=ones_t, in_=ones_f)
            # group-indicator matrices (host-built) for cross-partition
            # groupnorm reductions on PE
            Mg = singles.tile([128, CT, G], F32, tag="Mg")
            nc.sync.dma_start(out=Mg, in_=mg_d.rearrange("(t p) g -> p t g", p=128))
            M2 = singles.tile([G, CT, 128], F32, tag="M2")
            nc.sync.dma_start(out=M2, in_=m2_d.rearrange("g (t p) -> g t p", p=128))

            # ---- pass 1: groupnorm statistics over the full image ----
            stats_all = singles.tile([128, CT, MC, 6], F32, tag="stats_all")
            for mc in range(MC):
                x_mc = xs.tile([128, CT, 512], F32R, tag="xchunk")
                for t in range(CT):
                    nc.sync.dma_start(out=x_mc[:, t, :],
                                      in_=rx[:, t, mc * 512:(mc + 1) * 512])
                    nc.vector.bn_stats(out=stats_all[:, t, mc, :], in_=x_mc[:, t, :])
            mv = statp.tile([128, CT, 2], F32, tag="mv")
            for t in range(CT):
                nc.vector.bn_aggr(out=mv[:, t, :], in_=stats_all[:, t, :, :])
            # per-channel (mean, E[x^2])
            s_cat = statp.tile([128, CT, 2], F32, tag="s_cat")
            nc.vector.tensor_copy(out=s_cat[:, :, 0:1], in_=mv[:, :, 0:1])
            nc.vector.tensor_tensor(s_cat[:, :, 1:2], mv[:, :, 0:1], mv[:, :, 0:1], MULT)
            nc.vector.tensor_tensor(s_cat[:, :, 1:2], s_cat[:, :, 1:2], mv[:, :, 1:2], ADD)
            # cross-partition group reduction via PE: [32, 2] = Mg^T @ s_cat
            gsum_ps = ps_den.tile([G, 2], F32, tag="psden")
            for ct in range(CT):
                nc.tensor.matmul(gsum_ps, Mg[:, ct, :], s_cat[:, ct, :],
                                 start=(ct == 0), stop=(ct == CT - 1))
            gmean = statp.tile([G, 1], F32, tag="gmean")
            ge2 = statp.tile([G, 1], F32, tag="ge2")
            nc.vector.tensor_scalar_mul(gmean, gsum_ps[:, 0:1], 1.0 / GS)
            nc.vector.tensor_scalar_mul(ge2, gsum_ps[:, 1:2], 1.0 / GS)
            gvar = statp.tile([G, 1], F32, tag="gvar")
            nc.vector.tensor_tensor(gvar, gmean, gmean, MULT)
            nc.vector.tensor_tensor(gvar, ge2, gvar, SUB)
            grstd = statp.tile([G, 2], F32, tag="grstd")
            nc.scalar.activation(out=gvar, in_=gvar, func=SQRT, bias=eps_t, scale=1.0)
            nc.vector.reciprocal(grstd[:, 0:1], gvar)
            nc.vector.tensor_copy(out=grstd[:, 1:2], in_=gmean)
            # broadcast (rstd, mean) back to channel layout via PE: M2^T @ grstd
            rm_pc = statp.tile([128, CT, 2], F32, tag="rm_pc")
            for ct in range(CT):
                rm_ps = ps_den.tile([128, 2], F32, tag="psden")
                nc.tensor.matmul(rm_ps, M2[:, ct, :], grstd, start=True, stop=True)
                nc.vector.tensor_copy(out=rm_pc[:, ct, :], in_=rm_ps)
            # scale = gamma * rstd ; shift = beta - mean * scale   (channel layout)
            scale_pc = singles.tile([128, CT], F32, tag="scale_pc")
            shift_pc = singles.tile([128, CT], F32, tag="shift_pc")
            nc.vector.tensor_tensor(scale_pc, bias_cols["gamma"], rm_pc[:, :, 0], MULT)
            nc.vector.tensor_tensor(shift_pc, scale_pc, rm_pc[:, :, 1], MULT)
            nc.vector.tensor_tensor(shift_pc, bias_cols["beta"], shift_pc, SUB)

            def normalize(xt):
                for t in range(CT):
                    nc.vector.tensor_scalar(
                        xt[:, t, :], xt[:, t, :],
                        scale_pc[:, t:t + 1], shift_pc[:, t:t + 1], MULT, ADD)
                return xt

            # ---- weights: wqT early (q conv); others issued on gpsimd so they
            #      queue behind the stats chain and stay off the head DMA path ----
            wqT = kpool.tile([128, CT, C], F32R, tag="kmc")
            nc.scalar.dma_start(out=wqT, in_=wts["wqT"].rearrange("(t p) d -> p t d", p=128))

            # ---- q conv (full q for this core's chunk); also bank x+bo for the
            #      final residual while the raw chunks are on-chip ----
            q_sb = singles.tile([128, CT, NQ], F32R, tag="q_sb")
            xrb_all = singles.tile([128, CT, NQ], F32, tag="xrb_all")
            for nt in range(NT):
                ns = slice(nt * 512, (nt + 1) * 512)
                xq_mc = xs.tile([128, CT, 512], F32R, tag="xchunk")
                for t in range(CT):
                    nc.scalar.dma_start(out=xq_mc[:, t, :], in_=rxq[:, t, ns])
                    nc.vector.tensor_scalar(
                        xrb_all[:, t, ns], xq_mc[:, t, :],
                        bias_cols["bo"][:, t:t + 1], None, ADD)
                xq_mc = normalize(xq_mc)
                for dt in range(CT):
                    ps = ps_main.tile([128, 512], F32, tag="psmain")
                    for ct in range(CT):
                        nc.tensor.matmul(
                            ps, wqT[:, ct, dt * 128:(dt + 1) * 128],
                            xq_mc[:, ct, :],
                            start=(ct == 0), stop=(ct == CT - 1))
                    nc.vector.tensor_scalar(
                        q_sb[:, dt, ns], ps, inv_sqrt_c,
                        bias_cols["bqs"][:, dt:dt + 1], MULT, ADD)

            # ---- accumulators ----
            ao_acc = singles.tile([128, CT, NQ], F32R, tag="ao_acc")
            den_acc = singles.tile([1, NQ], F32, tag="den_acc")

            def touch(tile_ap):
                # Artificial WAW dep: keeps this tile's (prefetchable) DMA off
                # the head-critical DMA window until stats are done.
                nc.vector.tensor_copy(out=tile_ap[0:1, 0:1], in_=scale_pc[0:1, 0:1])

            wkT = wpool.tile([128, CT, C], F32R, tag="wkT")
            touch(wkT[:, 0])
            nc.scalar.dma_start(out=wkT, in_=wts["wkT"].rearrange("(t p) d -> p t d", p=128))
            wvT = wpool.tile([128, CT, C], F32R, tag="wvT")
            touch(wvT[:, 0])
            nc.scalar.dma_start(out=wvT, in_=wts["wvT"].rearrange("(t p) d -> p t d", p=128))
            woT = wpool.tile([128, CT, C], F32R, tag="woT")
            touch(woT[:, 0])
            nc.scalar.dma_start(out=woT, in_=wts["woT"].rearrange("(t p) d -> p t d", p=128))

            # ---- pass 2: flash-style main loop over m-chunks ----
            for mc in range(MC):
                ms_ = slice(mc * 512, (mc + 1) * 512)
                x_mc = xs.tile([128, CT, 512], F32R, tag="xchunk")
                touch(x_mc[:, 0])
                nc.sync.dma_start(out=x_mc, in_=rx[:, :, ms_])
                x_mc = normalize(x_mc)
                # k for this chunk: [c-part, ct, m]
                k_mc = kpool.tile([128, CT, C], F32R, tag="kmc")
                for dt in range(CT):
                    ps = ps_main.tile([128, 512], F32, tag="psmain")
                    for ct in range(CT):
                        nc.tensor.matmul(
                            ps, wkT[:, ct, dt * 128:(dt + 1) * 128],
                            x_mc[:, ct, :],
                            start=(ct == 0), stop=(ct == CT - 1))
                    nc.vector.tensor_scalar(
                        k_mc[:, dt, :], ps, bias_cols["bk"][:, dt:dt + 1], None, ADD)
                # vT for this chunk: [m-part(4x128), c]
                vT_mc = vpool.tile([128, 4, C], F32R, tag="vmc")
                for msub in range(4):
                    ps = ps_main.tile([128, 512], F32, tag="psmain")
                    for ct in range(CT):
                        nc.tensor.matmul(
                            ps, x_mc[:, ct, msub * 128:(msub + 1) * 128],
                            wvT[:, ct, :],
                            start=(ct == 0), stop=(ct == CT - 1))
                    nc.vector.tensor_tensor(vT_mc[:, msub, :], ps, bv_b, ADD)

                for nt in range(NT):
                    ns = slice(nt * 512, (nt + 1) * 512)
                    p_tiles = []
                    for msub in range(4):
                        ps_s = ps_main.tile([128, 512], F32, tag="psmain")
                        for ct in range(CT):
                            nc.tensor.matmul(
                                ps_s, k_mc[:, ct, msub * 128:(msub + 1) * 128],
                                q_sb[:, ct, ns],
                                start=(ct == 0), stop=(ct == CT - 1))
                        p_t = ppool.tile([128, 512], F32R, tag="pt")
                        nc.scalar.activation(out=p_t, in_=ps_s, func=EXP)
                        p_tiles.append(p_t)
                    # PV: accumulate the chunk in PSUM, then fold into SBUF accum
                    for ct in range(CT):
                        ps_o = ps_ao.tile([128, 512], F32, tag="psao")
                        for msub in range(4):
                            nc.tensor.matmul(
                                ps_o, vT_mc[:, msub, ct * 128:(ct + 1) * 128],
                                p_tiles[msub],
                                start=(msub == 0), stop=(msub == 3))
                        if mc == 0:
                            nc.vector.tensor_copy(out=ao_acc[:, ct, ns], in_=ps_o)
                        else:
                            nc.vector.tensor_tensor(
                                ao_acc[:, ct, ns], ao_acc[:, ct, ns], ps_o, ADD)
                    # softmax denominator: DVE-presum the 4 exp tiles, then a
                    # single ones-matmul (saves 3 full-stream M=1 matmuls on PE)
                    sum4 = opool.tile([128, 512], F32R, tag="ot")
                    nc.vector.tensor_tensor(sum4, p_tiles[0], p_tiles[1], ADD)
                    nc.vector.tensor_tensor(sum4, sum4, p_tiles[2], ADD)
                    nc.vector.tensor_tensor(sum4, sum4, p_tiles[3], ADD)
                    ps_d = ps_den.tile([1, 512], F32, tag="psden")
                    nc.tensor.matmul(ps_d, ones_t, sum4, start=True, stop=True)
                    if mc == 0:
                        nc.vector.tensor_copy(out=den_acc[:, ns], in_=ps_d)
                    else:
                        nc.vector.tensor_tensor(den_acc[:, ns], den_acc[:, ns], ps_d, ADD)

            # ---- proj (on unscaled ao; 1/den commutes through the matmul
            #      since it is per-column) + bias + residual ----
            for nt in range(NT):
                ns = slice(nt * 512, (nt + 1) * 512)
                den_b = xrpool.tile([128, 512], F32, tag="recb")
                nc.gpsimd.partition_broadcast(den_b, den_acc[:, ns])
                rec_b = xrpool.tile([128, 512], F32, tag="recb2")
                nc.vector.reciprocal(rec_b, den_b)
                for dt in range(CT):
                    ps = ps_main.tile([128, 512], F32, tag="psmain")
                    for ct in range(CT):
                        nc.tensor.matmul(
                            ps, woT[:, ct, dt * 128:(dt + 1) * 128],
                            ao_acc[:, ct, ns],
                            start=(ct == 0), stop=(ct == CT - 1))
                    o_t = opool.tile([128, 512], F32, tag="ot")
                    nc.vector.tensor_tensor(o_t, ps, rec_b, MULT)
                    nc.vector.tensor_tensor(o_t, o_t, xrb_all[:, dt, ns], ADD)
                    nc.scalar.dma_start(out=rout[:, dt, ns], in_=o_t)
    nc.finalize()
    return nc


_NC_CACHE = {}


def _get_nc(dt_mm=F32R):
    key = str(dt_mm)
    if key not in _NC_CACHE:
        _NC_CACHE[key] = _build(dt_mm)
    return _NC_CACHE[key]


def kernel(**inputs):
    x = np.ascontiguousarray(np.asarray(inputs["x"], dtype=np.float32))
    gamma = np.asarray(inputs["gamma"], np.float32)
    beta = np.asarray(inputs["beta"], np.float32)
    w = {n: np.asarray(inputs[n], np.float32) for n in ("wq", "wk", "wv", "wo")}
    b = {n: np.asarray(inputs[n], np.float32) for n in ("bq", "bk", "bv", "bo")}

    mg_np = np.zeros((C, G), np.float32)
    mg_np[np.arange(C), np.arange(C) // GS] = 1.0
    common = {
        "Mg": mg_np,
        "M2": np.ascontiguousarray(mg_np.T),
        "wqT": np.ascontiguousarray(w["wq"].T),
        "wkT": np.ascontiguousarray(w["wk"].T),
        "wvT": np.ascontiguousarray(w["wv"].T),
        "woT": np.ascontiguousarray(w["wo"].T),
        "bqs": (b["bq"] * (C ** -0.5)).astype(np.float32),
        "bk": b["bk"], "bv": b["bv"], "bo": b["bo"],
        "gamma": gamma, "beta": beta,
    }
    in_maps = []
    for core in range(N_CORES):
        bi, ch = divmod(core, 2)
        xi = x[bi].reshape(C, HW)
        m = dict(common)
        m["x_img"] = np.ascontiguousarray(xi)
        m["xq"] = np.ascontiguousarray(xi[:, ch * NQ:(ch + 1) * NQ])
        in_maps.append(m)

    nc = _get_nc()
    want_trace = bool(int(os.environ.get("KTRACE", "0")))
    if not want_trace:
        # The axon trace path needs antenv.axon_hooks, which this container
        # lacks; make sure an inherited BASS_TRACE can't route us there.
        os.environ["BASS_NEVER_TRACE"] = "1"
    global LAST_RESULTS
    LAST_RESULTS = run_bass_kernel_spmd(
        nc, in_maps, core_ids=list(range(N_CORES)), trace=want_trace,
    )
    full = np.empty((B, C, HW), np.float32)
    for core in range(N_CORES):
        bi, ch = divmod(core, 2)
        full[bi][:, ch * NQ:(ch + 1) * NQ] = LAST_RESULTS.results[core]["out"]
    return full.reshape(B, C, H, W)


# revision 27
# speedup vs baseline: 1.0051x; 1.0051x over previous
"""AttnBlock (GroupNorm -> q/k/v 1x1 conv -> spatial softmax attention -> proj -> residual)
for Trainium2, 8 NeuronCores.

Sharding: core i handles batch i//2, query-position chunk i%2 (2048 of 4096 positions).
Each core receives the full image of its batch (needed for GroupNorm stats and full K/V),
computes K/V for all positions (2x duplicated work, ~10% overhead, no collectives needed),
and attention rows for its own query chunk.

Kernel structure (per core), all matmuls in float32r (1 cyc/row at free>=256):
  pass 1: stream x in 512-wide chunks -> bn_stats; cross-partition group reduction
          and group->channel broadcast via tiny PE matmuls against 0/1 indicator
          matrices (no DRAM round trips on the critical path).
  q conv: stream xq chunks, normalize, q = wqT^T @ h_q (scaled by c^-0.5, biased).
  pass 2 (flash-style, k/v never fully materialized): for each 512-wide m-chunk:
          normalize -> k_mc, vT_mc convs; for each 512-wide n-tile:
          sT[m,n] = k^T q (PSUM), P = exp(sT) (ScalarE, PSUM->SBUF),
          PV partial = vT^T @ P accumulated in PSUM over the chunk then added to
          an SBUF accumulator; softmax denominator = DVE-presum of the 4 exp
          tiles followed by a single ones-matmul on PE.
  final:  proj conv runs on the unscaled accumulator (the per-column 1/den commutes
          through the matmul); epilogue = po * (1/den, partition-broadcast) + (x + bo)
          banked in SBUF during the q-conv pass; DMA out.
"""

import math
import os
import sys

sys.path.insert(0, "/opt/trn_rl_repo")

import numpy as np

import concourse.bacc as bacc
import concourse.bass as bass
import concourse.mybir as mybir
import concourse.tile as tile
from concourse.bass_utils import run_bass_kernel_spmd

F32 = mybir.dt.float32
F32R = mybir.dt.float32r
MULT = mybir.AluOpType.mult
ADD = mybir.AluOpType.add
SUB = mybir.AluOpType.subtract
AX = mybir.AxisListType.X
XY = mybir.AxisListType.XY
EXP = mybir.ActivationFunctionType.Exp
SQRT = mybir.ActivationFunctionType.Sqrt

B, C, H, W = 4, 512, 64, 64
HW = H * W              # 4096
G = 32                  # groups
GS = C // G             # 16 channels per group
NQ = HW // 2            # query positions per core
EPS = 1e-5
N_CORES = 8

LAST_RESULTS = None     # BassKernelResults of the most recent run (for profiling)


def _build(dt_mm=F32R):
    CT = C // 128            # 4 channel partition-tiles
    NT = NQ // 512           # 4 n-tiles per core
    MC = HW // 512           # 8 m-chunks
    inv_sqrt_c = 1.0 / math.sqrt(C)

    nc = bacc.Bacc("TRN2", target_bir_lowering=False, debug=False)

    x_img = nc.dram_tensor("x_img", [C, HW], F32R, kind="ExternalInput").ap()
    xq = nc.dram_tensor("xq", [C, NQ], F32R, kind="ExternalInput").ap()
    wts = {
        n: nc.dram_tensor(n, [C, C], F32R, kind="ExternalInput").ap()
        for n in ("wqT", "wkT", "wvT", "woT")
    }
    mg_d = nc.dram_tensor("Mg", [C, G], F32, kind="ExternalInput").ap()
    m2_d = nc.dram_tensor("M2", [G, C], F32, kind="ExternalInput").ap()
    vecs = {
        n: nc.dram_tensor(n, [C], F32, kind="ExternalInput").ap()
        for n in ("bqs", "bk", "bv", "bo", "gamma", "beta")
    }
    out = nc.dram_tensor("out", [C, NQ], F32, kind="ExternalOutput").ap()

    rx = x_img.rearrange("(t p) m -> p t m", p=128)
    rxq = xq.rearrange("(t p) n -> p t n", p=128)
    rout = out.rearrange("(t p) n -> p t n", p=128)

    with tile.TileContext(nc) as tc:
        with (
            tc.tile_pool(name="singles", bufs=1) as singles,
            tc.tile_pool(name="wpool", bufs=1) as wpool,
            tc.tile_pool(name="xs", bufs=2) as xs,
            tc.tile_pool(name="statp", bufs=2) as statp,
            tc.tile_pool(name="kpool", bufs=2) as kpool,
            tc.tile_pool(name="vpool", bufs=2) as vpool,
            tc.tile_pool(name="ppool", bufs=5) as ppool,
            tc.tile_pool(name="opool", bufs=3) as opool,
            tc.tile_pool(name="xrpool", bufs=2) as xrpool,
            tc.tile_pool(name="ps_main", bufs=3, space="PSUM") as ps_main,
            tc.tile_pool(name="ps_ao", bufs=4, space="PSUM") as ps_ao,
            tc.tile_pool(name="ps_den", bufs=1, space="PSUM") as ps_den,
            tc.tile_pool(name="dram", bufs=1, space="DRAM") as dram,
        ):
            # ---- constants / small loads ----
            bias_cols = {}
            for n in ("bqs", "bk", "bo", "gamma", "beta"):
                t = singles.tile([128, CT], F32, tag=f"col_{n}")
                nc.sync.dma_start(out=t, in_=vecs[n].rearrange("(t p) -> p t", p=128))
                bias_cols[n] = t
            # bv replicated across all 128 partitions (vT has m on partitions)
            bv_b = singles.tile([128, C], F32, tag="bv_b")
            bv_src = vecs["bv"]
            nc.sync.dma_start(
                out=bv_b,
                in_=bass.AP(tensor=bv_src.tensor, offset=bv_src.offset,
                            ap=[[0, 128], bv_src.ap[0]]),
            )
            eps_t = singles.tile([G, 1], F32, tag="eps")
            nc.vector.memset(eps_t, EPS)
            ones_f = singles.tile([128, 1], F32, tag="ones_f")
            nc.vector.memset(ones_f, 1.0)
            dummy = singles.tile([1, 1], F32, tag="dummy")
            nc.vector.memset(dummy, 1.0)
            nc.scalar.activation(out=dummy, in_=dummy, func=SQRT)
            nc.scalar.activation(out=dummy, in_=dummy, func=EXP)
            ones_t = singles.tile([128, 1], F32R, tag="ones")
            nc.vector.tensor_copy(out=ones_t, in_=ones_f)
            # group-indicator matrices (host-built) for cross-partition
            # groupnorm reductions on PE
            Mg = singles.tile([128, CT, G], F32, tag="Mg")
            nc.sync.dma_start(out=Mg, in_=mg_d.rearrange("(t p) g -> p t g", p=128))
            M2 = singles.tile([G, CT, 128], F32, tag="M2")
            nc.sync.dma_start(out=M2, in_=m2_d.rearrange("g (t p) -> g t p", p=128))

            # ---- pass 1: groupnorm statistics over the full image ----
            stats_all = singles.tile([128, CT, MC, 6], F32, tag="stats_all")
            for mc in range(MC):
                x_mc = xs.tile([128, CT, 512], F32R, tag="xchunk")
                for t in range(CT):
                    nc.sync.dma_start(out=x_mc[:, t, :],
                                      in_=rx[:, t, mc * 512:(mc + 1) * 512])
                    nc.vector.bn_stats(out=stats_all[:, t, mc, :], in_=x_mc[:, t, :])
            mv = statp.tile([128, CT, 2], F32, tag="mv")
            for t in range(CT):
                nc.vector.bn_aggr(out=mv[:, t, :], in_=stats_all[:, t, :, :])
            # per-channel (mean, E[x^2])
            s_cat = statp.tile([128, CT, 2], F32, tag="s_cat")
            nc.vector.tensor_copy(out=s_cat[:, :, 0:1], in_=mv[:, :, 0:1])
            nc.vector.tensor_tensor(s_cat[:, :, 1:2], mv[:, :, 0:1], mv[:, :, 0:1], MULT)
            nc.vector.tensor_tensor(s_cat[:, :, 1:2], s_cat[:, :, 1:2], mv[:, :, 1:2], ADD)
            # cross-partition group reduction via PE: [32, 2] = Mg^T @ s_cat
            gsum_ps = ps_den.tile([G, 2], F32, tag="psden")
            for ct in range(CT):
                nc.tensor.matmul(gsum_ps, Mg[:, ct, :], s_cat[:, ct, :],
                                 start=(ct == 0), stop=(ct == CT - 1))
            gmean = statp.tile([G, 1], F32, tag="gmean")
            ge2 = statp.tile([G, 1], F32, tag="ge2")
            nc.vector.tensor_scalar_mul(gmean, gsum_ps[:, 0:1], 1.0 / GS)
            nc.vector.tensor_scalar_mul(ge2, gsum_ps[:, 1:2], 1.0 / GS)
            gvar = statp.tile([G, 1], F32, tag="gvar")
            nc.vector.tensor_tensor(gvar, gmean, gmean, MULT)
            nc.vector.tensor_tensor(gvar, ge2, gvar, SUB)
            grstd = statp.tile([G, 2], F32, tag="grstd")
            nc.scalar.activation(out=gvar, in_=gvar, func=SQRT, bias=eps_t, scale=1.0)
            nc.vector.reciprocal(grstd[:, 0:1], gvar)
            nc.vector.tensor_copy(out=grstd[:, 1:2], in_=gmean)
            # broadcast (rstd, mean) back to channel layout via PE: M2^T @ grstd
            rm_pc = statp.tile([128, CT, 2], F32, tag="rm_pc")
            for ct in range(CT):
                rm_ps = ps_den.tile([128, 2], F32, tag="psden")
                nc.tensor.matmul(rm_ps, M2[:, ct, :], grstd, start=True, stop=True)
                nc.vector.tensor_copy(out=rm_pc[:, ct, :], in_=rm_ps)
            # scale = gamma * rstd ; shift = beta - mean * scale   (channel layout)
            scale_pc = singles.tile([128, CT], F32, tag="scale_pc")
            shift_pc = singles.tile([128, CT], F32, tag="shift_pc")
            nc.vector.tensor_tensor(scale_pc, bias_cols["gamma"], rm_pc[:, :, 0], MULT)
            nc.vector.tensor_tensor(shift_pc, scale_pc, rm_pc[:, :, 1], MULT)
            nc.vector.tensor_tensor(shift_pc, bias_cols["beta"], shift_pc, SUB)

            def normalize(xt):
                for t in range(CT):
                    nc.vector.tensor_scalar(
                        xt[:, t, :], xt[:, t, :],
                        scale_pc[:, t:t + 1], shift_pc[:, t:t + 1], MULT, ADD)
                return xt

            # ---- weights: wqT early (q conv); others issued on gpsimd so they
            #      queue behind the stats chain and stay off the head DMA path ----
            wqT = kpool.tile([128, CT, C], F32R, tag="kmc")
            nc.scalar.dma_start(out=wqT, in_=wts["wqT"].rearrange("(t p) d -> p t d", p=128))

            # ---- q conv (full q for this core's chunk); also bank x+bo for the
            #      final residual while the raw chunks are on-chip ----
            q_sb = singles.tile([128, CT, NQ], F32R, tag="q_sb")
            xrb_all = singles.tile([128, CT, NQ], F32, tag="xrb_all")
            for nt in range(NT):
                ns = slice(nt * 512, (nt + 1) * 512)
                xq_mc = xs.tile([128, CT, 512], F32R, tag="xchunk")
                for t in range(CT):
                    nc.scalar.dma_start(out=xq_mc[:, t, :], in_=rxq[:, t, ns])
                    nc.gpsimd.tensor_scalar(
                        xrb_all[:, t, ns], xq_mc[:, t, :],
                        bias_cols["bo"][:, t:t + 1], None, ADD)
                xq_mc = normalize(xq_mc)
                for dt in range(CT):
                    ps = ps_main.tile([128, 512], F32, tag="psmain")
                    for ct in range(CT):
                        nc.tensor.matmul(
                            ps, wqT[:, ct, dt * 128:(dt + 1) * 128],
                            xq_mc[:, ct, :],
                            start=(ct == 0), stop=(ct == CT - 1))
                    nc.vector.tensor_scalar(
                        q_sb[:, dt, ns], ps, inv_sqrt_c,
                        bias_cols["bqs"][:, dt:dt + 1], MULT, ADD)

            # ---- accumulators ----
            ao_acc = singles.tile([128, CT, NQ], F32R, tag="ao_acc")
            den_acc = singles.tile([1, NQ], F32, tag="den_acc")

            def touch(tile_ap):
                # Artificial WAW dep: keeps this tile's (prefetchable) DMA off
                # the head-critical DMA window until stats are done.
                nc.vector.tensor_copy(out=tile_ap[0:1, 0:1], in_=scale_pc[0:1, 0:1])

            wkT = wpool.tile([128, CT, C], F32R, tag="wkT")
            touch(wkT[:, 0])
            nc.scalar.dma_start(out=wkT, in_=wts["wkT"].rearrange("(t p) d -> p t d", p=128))
            wvT = wpool.tile([128, CT, C], F32R, tag="wvT")
            touch(wvT[:, 0])
            nc.scalar.dma_start(out=wvT, in_=wts["wvT"].rearrange("(t p) d -> p t d", p=128))
            woT = wpool.tile([128, CT, C], F32R, tag="woT")
            touch(woT[:, 0])
            nc.scalar.dma_start(out=woT, in_=wts["woT"].rearrange("(t p) d -> p t d", p=128))

            # ---- pass 2: flash-style main loop over m-chunks ----
            for mc in range(MC):
                ms_ = slice(mc * 512, (mc + 1) * 512)
                x_mc = xs.tile([128, CT, 512], F32R, tag="xchunk")
                touch(x_mc[:, 0])
                nc.sync.dma_start(out=x_mc, in_=rx[:, :, ms_])
                x_mc = normalize(x_mc)
                # k for this chunk: [c-part, ct, m]
                k_mc = kpool.tile([128, CT, C], F32R, tag="kmc")
                for dt in range(CT):
                    ps = ps_main.tile([128, 512], F32, tag="psmain")
                    for ct in range(CT):
                        nc.tensor.matmul(
                            ps, wkT[:, ct, dt * 128:(dt + 1) * 128],
                            x_mc[:, ct, :],
                            start=(ct == 0), stop=(ct == CT - 1))
                    nc.vector.tensor_scalar(
                        k_mc[:, dt, :], ps, bias_cols["bk"][:, dt:dt + 1], None, ADD)
                # vT for this chunk: [m-part(4x128), c]
                vT_mc = vpool.tile([128, 4, C], F32R, tag="vmc")
                for msub in range(4):
                    ps = ps_main.tile([128, 512], F32, tag="psmain")
                    for ct in range(CT):
                        nc.tensor.matmul(
                            ps, x_mc[:, ct, msub * 128:(msub + 1) * 128],
                            wvT[:, ct, :],
                            start=(ct == 0), stop=(ct == CT - 1))
                    nc.vector.tensor_tensor(vT_mc[:, msub, :], ps, bv_b, ADD)

                for nt in range(NT):
                    ns = slice(nt * 512, (nt + 1) * 512)
                    p_tiles = []
                    for msub in range(4):
                        ps_s = ps_main.tile([128, 512], F32, tag="psmain")
                        for ct in range(CT):
                            nc.tensor.matmul(
                                ps_s, k_mc[:, ct, msub * 128:(msub + 1) * 128],
                                q_sb[:, ct, ns],
                                start=(ct == 0), stop=(ct == CT - 1))
                        p_t = ppool.tile([128, 512], F32R, tag="pt")
                        nc.scalar.activation(out=p_t, in_=ps_s, func=EXP)
                        p_tiles.append(p_t)
                    # PV: accumulate the chunk in PSUM, then fold into SBUF accum
                    for ct in range(CT):
                        ps_o = ps_ao.tile([128, 512], F32, tag="psao")
                        for msub in range(4):
                            nc.tensor.matmul(
                                ps_o, vT_mc[:, msub, ct * 128:(ct + 1) * 128],
                                p_tiles[msub],
                                start=(msub == 0), stop=(msub == 3))
                        if mc == 0:
                            nc.vector.tensor_copy(out=ao_acc[:, ct, ns], in_=ps_o)
                        else:
                            nc.vector.tensor_tensor(
                                ao_acc[:, ct, ns], ao_acc[:, ct, ns], ps_o, ADD)
                    # softmax denominator: DVE-presum the 4 exp tiles, then a
                    # single ones-matmul (saves 3 full-stream M=1 matmuls on PE)
                    sum4 = opool.tile([128, 512], F32R, tag="ot")
                    nc.vector.tensor_tensor(sum4, p_tiles[0], p_tiles[1], ADD)
                    nc.vector.tensor_tensor(sum4, sum4, p_tiles[2], ADD)
                    nc.vector.tensor_tensor(sum4, sum4, p_tiles[3], ADD)
                    ps_d = ps_den.tile([1, 512], F32, tag="psden")
                    nc.tensor.matmul(ps_d, ones_t, sum4, start=True, stop=True)
                    if mc == 0:
                        nc.vector.tensor_copy(out=den_acc[:, ns], in_=ps_d)
                    else:
                        nc.vector.tensor_tensor(den_acc[:, ns], den_acc[:, ns], ps_d, ADD)

            # ---- proj (on unscaled ao; 1/den commutes through the matmul
            #      since it is per-column) + bias + residual ----
            for nt in range(NT):
                ns = slice(nt * 512, (nt + 1) * 512)
                den_b = xrpool.tile([128, 512], F32, tag="recb")
                nc.gpsimd.partition_broadcast(den_b, den_acc[:, ns])
                rec_b = xrpool.tile([128, 512], F32, tag="recb2")
                nc.vector.reciprocal(rec_b, den_b)
                for dt in range(CT):
                    ps = ps_main.tile([128, 512], F32, tag="psmain")
                    for ct in range(CT):
                        nc.tensor.matmul(
                            ps, woT[:, ct, dt * 128:(dt + 1) * 128],
                            ao_acc[:, ct, ns],
                            start=(ct == 0), stop=(ct == CT - 1))
                    o_t = opool.tile([128, 512], F32, tag="ot")
                    nc.vector.tensor_tensor(o_t, ps, rec_b, MULT)
                    nc.vector.tensor_tensor(o_t, o_t, xrb_all[:, dt, ns], ADD)
                    nc.scalar.dma_start(out=rout[:, dt, ns], in_=o_t)
    nc.finalize()
    return nc


_NC_CACHE = {}


def _get_nc(dt_mm=F32R):
    key = str(dt_mm)
    if key not in _NC_CACHE:
        _NC_CACHE[key] = _build(dt_mm)
    return _NC_CACHE[key]


def kernel(**inputs):
    x = np.ascontiguousarray(np.asarray(inputs["x"], dtype=np.float32))
    gamma = np.asarray(inputs["gamma"], np.float32)
    beta = np.asarray(inputs["beta"], np.float32)
    w = {n: np.asarray(inputs[n], np.float32) for n in ("wq", "wk", "wv", "wo")}
    b = {n: np.asarray(inputs[n], np.float32) for n in ("bq", "bk", "bv", "bo")}

    mg_np = np.zeros((C, G), np.float32)
    mg_np[np.arange(C), np.arange(C) // GS] = 1.0
    common = {
        "Mg": mg_np,
        "M2": np.ascontiguousarray(mg_np.T),
        "wqT": np.ascontiguousarray(w["wq"].T),
        "wkT": np.ascontiguousarray(w["wk"].T),
        "wvT": np.ascontiguousarray(w["wv"].T),
        "woT": np.ascontiguousarray(w["wo"].T),
        "bqs": (b["bq"] * (C ** -0.5)).astype(np.float32),
        "bk": b["bk"], "bv": b["bv"], "bo": b["bo"],
        "gamma": gamma, "beta": beta,
    }
    in_maps = []
    for core in range(N_CORES):
        bi, ch = divmod(core, 2)
        xi = x[bi].reshape(C, HW)
        m = dict(common)
        m["x_img"] = np.ascontiguousarray(xi)
        m["xq"] = np.ascontiguousarray(xi[:, ch * NQ:(ch + 1) * NQ])
        in_maps.append(m)

    nc = _get_nc()
    want_trace = bool(int(os.environ.get("KTRACE", "0")))
    if not want_trace:
        # The axon trace path needs antenv.axon_hooks, which this container
        # lacks; make sure an inherited BASS_TRACE can't route us there.
        os.environ["BASS_NEVER_TRACE"] = "1"
    global LAST_RESULTS
    LAST_RESULTS = run_bass_kernel_spmd(
        nc, in_maps, core_ids=list(range(N_CORES)), trace=want_trace,
    )
    full = np.empty((B, C, HW), np.float32)
    for core in range(N_CORES):
        bi, ch = divmod(core, 2)
        full[bi][:, ch * NQ:(ch + 1) * NQ] = LAST_RESULTS.results[core]["out"]
    return full.reshape(B, C, H, W)
